# revision 48
# baseline (speedup 1.0000x reference)
"""ExpertsChooseMlp Trainium2 kernel — all-fp8 DoubleRow pipeline.

Full inputs in, full output out. Sharding: 8 cores = 4 batches x 2 expert-pairs.
Core m handles batch b=m//2 and experts {2g, 2g+1}, g=m%2. Each core computes
pout[T,O] = sum_{e in pair} combine[b,:,e,:] @ mlp_e(dispatch[b,:,e,:]^T @ x[b]);
the host sums the two partials per batch and adds b2 + rank-1 corrections.

All four matmul phases run as fp8-e4m3 DoubleRow (K=256/pass, ~1.75x bf16):
  xdT[D,C] = matmul(lhsT=x8[T,D],   rhs=u8[T,C],  DR)   (K=T)
  hT[HE,C] = matmul(lhsT=w18[D,HE], rhs=xdT[D,C], DR)   (K=D), GELU + beta
  y[C,O]   = matmul(lhsT=hT[HE,C],  rhs=w28[HE,O],DR)   (K=HE)
  pout[T,O]= matmul(lhsT=cmT[C,T],  rhs=y[C,O],   DR)   (K=C)

fp8 error control (fp32 reference absmax-rel err ~8.7e-3, budget 2e-2):
fp8 quantization errors of per-(t,c) data average out across the capacity
reduction, but errors in operands SHARED across C (x, w1, w2) are correlated
and do not. Three rank-1 corrections cancel those correlated components:
  1. dispatch mask is mean-shifted: u = dm - 0.5 quantized to fp8 (halves the
     mask quant error); the c-independent term v[d] = 0.5*sum_t x[t,d] is
     computed host-side from EXACT fp32 x, killing the rank-1 part of x's
     quant error.
  2. v is folded through fc1 with EXACT fp32 w1: beta = w1^T v + b1 becomes
     the per-partition activation bias, killing the rank-1 part of w1's
     quant error.
  3. w2's correlated error mean_c(h)^T (w2 - w28) is cancelled host-side
     with mu_h = E_c[h] evaluated analytically (pre-GELU activations are
     ~N(beta, s2) across c; E[gelu(N(beta,s2))] has a closed form), adding
     outer(rowsum_cm, mu_h @ ew2) to the output.

Schedule notes (trace-driven; measured 87.2-89.0us HW exec across runs,
vs 128.9us for the bf16 baseline and a ~69.1us pure-matmul floor at the
fp8 peak; dominant fixed costs outside the matmul stream: ~3.9us head
[framework preamble barrier -> first dm tile landed] and ~11.5us tail
[last-DMA completion-sem latency ~1.3us + a compiler-emitted NEFF
epilogue that zeroes all 253 semaphores one instruction each, ~6us,
plus exit barriers/notify]):
  - fp8 DR matmul streams N=512 in ~216ns (the 157 TF/s fp8 peak);
    LDWEIGHTS fully hides under the previous matmul's streaming.
  - All HBM operands are pre-arranged host-side into the exact SBUF
    DoubleRow plane layouts, so every DMA is a big contiguous burst
    (strided gather descriptors measured only ~25-35 GB/s/queue).
    dm carries an extra ncc-half split so each half is one contiguous
    128KB transfer.
  - Only 2 HWDGE queues exist (sync + scalar engines ring their paced
    doorbells, which OCCUPIES the engine until its transfers finish);
    x/dm interleave across both in consumption order — one queue cannot
    sustain phase A's 148 GB/s arrival rate during the DMA ramp (4us of
    A0 stalls when dm-e0 was single-queue).  Entry SIZING beats
    fine-grained JIT: each queue overlaps ~4 in-flight entries and loses
    ~0.3-0.7us per entry to completion-sem gaps, so all-halved dm
    (32 entries) starved the dm-e1 tail (~2us of A1 stalls).  Only dm
    tiles (e0,kp0..1) are h-split — their halves gate the stream start
    and passes 0-1 during the doorbell-limited ramp; everything else
    ships as 256KB entries.  Weights load after the dm streams (B0/C0
    deadlines are late); cmt queues last (needed only by phase D ~52us).
  - Stage order A0,A1,B0,C0,B1,C1,D.  A0 runs kp-major over all 8 PSUM
    banks (dm tiles consumed JIT as they stream); its PSUM->SBUF casts
    alternate vector/scalar in BSEQ bank-stop order (serial vector-only
    casts trailed A1's diagonal ramp by 0.7-2.4us EVERY run; the scalar
    engine is only free for them because the cmt doorbells are emitted
    after stage_A(0) in program order).  A1 runs a (kp+bank)
    diagonal wavefront: bank j's chain starts as soon as A0's cast j
    freed that bank, and wave s needs only dm-e1 tiles 0..s — measured
    0.8-1.5us residual boundary stall vs 2.4us for kp-major (the
    scheduler otherwise reorders A1 bank-major and stalls on the dm-e1
    arrival tail).  Full round-splitting (ncc rounds) was NET WORSE: it
    needs h0-first DMA order, which start-gates the stream (see above).
  - pout is written bf16 (f32's 4MB cannot drain inside phase D on one
    ~125 GB/s write queue; splitting across queues contends DOWN), with
    drain copies alternating scalar/vector (vector is busy with C1's
    y-copies when the drain starts); mt13 ships on the scalar queue and
    the final chunk is column-split vector/scalar so each 64KB half hits
    an idle queue as soon as its copy lands.
  - Warmup: 5 bf16 N=512 matmuls off a vector memset, sized to end at
    the MEDIAN first-dm-half completion (~10.1us; measured 9.7-11.0).
    HAM un-throttles ~3.4-4us after SUSTAINED PE activity starts, and a
    PE idle gap between warmup end and data-ready resets its
    busy-window tracking (HAM then fired 14-17us instead of ~11, i.e.
    6-10 cold 1.2GHz DR matmuls instead of ~2 — measured both ways).
    A 30x N=64 warmup chain measured a chip-wide 5/6 downclock (P0
    power state: every engine 1.2x slower, DMA unaffected, ~+16us) —
    do not use many small-N matmuls here.  gpsimd cannot read PSUM,
    and its SWDGE steals HBM bandwidth if used during the critical dm
    window.  Dummy first-queue entries do NOT absorb the ~1.5us
    doorbell->wire latency (it is per-entry, not queue spin-up).
"""
import sys

sys.path.insert(0, "/opt/trn_rl_repo")

import numpy as np
import ml_dtypes

import concourse.bacc as bacc
import concourse.mybir as mybir
import concourse.tile as tile
from concourse import bass_utils

B, T, D, E, C, HE, O = 4, 2048, 512, 4, 1024, 512, 512
P = 128
nKP = T // (2 * P)   # 8  T pair-chunks (DR: K=256 per matmul)
nMD = D // P         # 4  D-chunks
nMH = HE // P        # 4  HE-chunks
nKD2 = D // (2 * P)  # 2  D pair-chunks
nCC = C // P         # 8  C-chunks
nKH2 = HE // (2 * P) # 2  HE pair-chunks
nMT = T // P         # 16
NF = 512             # matmul free dim (one PSUM bank)
nCP = nCC // 2       # 4  C pair-chunks for combine

F32 = mybir.dt.float32
BF16 = mybir.dt.bfloat16
F8 = mybir.dt.float8e4
GELU = mybir.ActivationFunctionType.Gelu
DR = mybir.MatmulPerfMode.DoubleRow
DM_SHIFT = 0.5

_NC = None


def _build():
    nc = bacc.Bacc("TRN2", target_bir_lowering=False, debug=False,
                   enable_asserts=False, num_devices=1)
    # All inputs pre-arranged host-side into SBUF plane layouts (contiguous).
    # dm tiles (e0,kp0..1) ship h-split via dm0 [2(tile), 2(ncc), P, 2, NF]
    # (each 128KB half one contiguous DMA — they gate the stream start and
    # the first kp passes); the other 14 tiles ship whole via dmr (256KB
    # entries have ~15% better effective queue throughput than halves).
    # x is partition-major = x_sb's exact SBUF layout.
    xb = nc.dram_tensor("xb", [P, nKP, 2, D], F8, kind="ExternalInput").ap()
    dm0 = nc.dram_tensor("dm0", [2, 2, P, 2, NF], F8, kind="ExternalInput").ap()
    dmr = nc.dram_tensor("dmr", [2 * nKP - 2, P, 2, C], F8,
                         kind="ExternalInput").ap()
    cmt = nc.dram_tensor("cmt", [2, nCP, P, 2, T], F8, kind="ExternalInput").ap()
    w1 = nc.dram_tensor("w1", [P, 2, nKD2, 2, HE], F8, kind="ExternalInput").ap()
    w2 = nc.dram_tensor("w2", [P, 2, nKH2, 2, O], F8, kind="ExternalInput").ap()
    beta = nc.dram_tensor("beta", [P, 2 * nMH], F32, kind="ExternalInput").ap()
    # pout in bf16: one HWDGE queue writes ~125 GB/s (splitting across queues
    # contends DOWN to ~105), so f32's 4MB can't drain inside phase D's 27us
    # — bf16's 2MB can. Costs ~+3e-3 absmax-rel worst case.
    pout = nc.dram_tensor("pout", [T, O], BF16, kind="ExternalOutput").ap()

    with tile.TileContext(nc) as tc:
        with (
            tc.tile_pool(name="const", bufs=1) as const,
            tc.tile_pool(name="dmp", bufs=16) as dmp,
            tc.tile_pool(name="cmp", bufs=8) as cmp_,
            tc.tile_pool(name="inter", bufs=2) as inter,
            tc.tile_pool(name="yp", bufs=2) as yp,
            tc.tile_pool(name="outp", bufs=2) as outp,
            tc.tile_pool(name="psum", bufs=8, space="PSUM") as psp,
        ):
            # ---- DMA plan (2 HWDGE queues, deadline-ordered) ----
            # Entry sizing matters more than fine-grained JIT: each queue
            # overlaps ~4 in-flight entries and loses ~0.3-0.7us per entry
            # to completion gaps, so 32 half-tile dm entries measured ~15%
            # less effective throughput than 256KB entries — which starved
            # the dm-e1 tail right when A1's diagonal needed it.  Only
            # tile (e0,kp0) is h-split (its h0 half start-gates the whole
            # matmul stream); everything else ships as big contiguous
            # entries, deadline-ordered and alternating across queues.
            x_sb = const.tile([P, nKP, 2, D], F8)
            dm_t = {}
            for kp in range(2):
                dm_t[(0, kp)] = dmp.tile([P, 2, 2, NF], F8, tag="dm",
                                         name=f"dm_t0{kp}s")
            for j in range(2 * nKP - 2):
                ei, kp = (0, j + 2) if j < nKP - 2 else (1, j - (nKP - 2))
                dm_t[(ei, kp)] = dmp.tile([P, 2, C], F8, tag="dm",
                                          name=f"dm_t{ei}{kp}")
            nc.sync.dma_start(dm_t[(0, 0)][:, 0], dm0[0, 0])
            nc.scalar.dma_start(x_sb[:, 0, :, :], xb[:, 0])
            nc.sync.dma_start(dm_t[(0, 0)][:, 1], dm0[0, 1])
            nc.scalar.dma_start(x_sb[:, 1, :, :], xb[:, 1])
            nc.sync.dma_start(dm_t[(0, 1)][:, 1], dm0[1, 1])
            nc.scalar.dma_start(dm_t[(0, 1)][:, 0], dm0[1, 0])
            nc.sync.dma_start(dm_t[(0, 2)][:], dmr[0])
            nc.scalar.dma_start(x_sb[:, 2, :, :], xb[:, 2])
            nc.sync.dma_start(x_sb[:, 3, :, :], xb[:, 3])
            nc.scalar.dma_start(dm_t[(0, 3)][:], dmr[1])
            nc.sync.dma_start(dm_t[(0, 4)][:], dmr[2])
            nc.scalar.dma_start(x_sb[:, 4, :, :], xb[:, 4])
            nc.sync.dma_start(x_sb[:, 5, :, :], xb[:, 5])
            nc.scalar.dma_start(dm_t[(0, 5)][:], dmr[3])
            nc.sync.dma_start(dm_t[(0, 6)][:], dmr[4])
            nc.scalar.dma_start(x_sb[:, 6, :, :], xb[:, 6])
            nc.sync.dma_start(x_sb[:, 7, :, :], xb[:, 7])
            nc.scalar.dma_start(dm_t[(0, 7)][:], dmr[5])
            for kp in range(nKP):
                eng = nc.scalar if kp % 2 == 0 else nc.sync
                eng.dma_start(dm_t[(1, kp)][:], dmr[nKP - 2 + kp])
            w1_sb = const.tile([P, 2, nKD2, 2, HE], F8)
            nc.sync.dma_start(w1_sb[:], w1[:])
            beta_sb = const.tile([P, 2 * nMH], F32)
            nc.sync.dma_start(beta_sb[:], beta[:])
            w2_sb = const.tile([P, 2, nKH2, 2, O], F8)
            nc.scalar.dma_start(w2_sb[:], w2[:])
            cmt_t = {}
            for ei in range(2):
                for kp in range(nCP):
                    t_ = cmp_.tile([P, 2, T], F8, tag="cmt")
                    cmt_t[(ei, kp)] = t_
            # cmt dma_starts are emitted AFTER stage_A(0): their doorbells
            # would otherwise sit ahead of A0's scalar-side casts in the
            # scalar engine's stream and push them past 30us (doorbells
            # occupy the engine until flow control clears).  cmt has ~12us
            # of slack (needed ~52us, lands ~41us even when issued there).

            # ---- HAM warmup: 5 bf16 matmuls on a vector-memset tile during
            # the initial DMA wait.  Sized to end ~10.1us = near the MEDIAN
            # first-dm-half completion (measured 9.7-11.0): a PE idle gap
            # between warmup end and data-ready resets HAM's busy-window
            # tracking (HAM then fired at 14-17us instead of ~11.5, i.e.
            # 6-10 cold DR matmuls instead of ~2).  (A 30x N=64 chain
            # measured a chip-wide 2.0 GHz power-state downclock — do not
            # use many small-N matmuls here.)
            warm = const.tile([P, NF], BF16)
            nc.vector.memset(warm[:], 0.0)
            ps_w = psp.tile([P, NF], F32, tag="ps", name="ps_warm")
            for i in range(5):
                nc.tensor.matmul(ps_w[:], warm[:, 0:P], warm[:],
                                 start=(i == 0), stop=(i == 4))

            xdt = {}
            # bank b = 2*mc + ncc; BSEQ = bank completion order of the
            # ncc-major kp pass (= CAST emission order = the next phase's
            # bank-free order).
            BSEQ = [0, 2, 4, 6, 1, 3, 5, 7]

            def stage_A(ei, diagonal):
                # xdT[D, C] = x8^T u8, fp8 DR, all 8 PSUM banks.
                # A0 runs kp-major: each dm tile is consumed right as it
                # streams in (DMA-JIT at the ramp).  A1 runs a (kp+bank)
                # diagonal wavefront: bank j's chain starts as soon as A0's
                # CAST j freed that PSUM bank, and wave s only needs dm-e1
                # tiles 0..s — so neither the CAST cadence nor the dm-e1
                # arrival tail stalls the PE (measured 0.8us residual at
                # the A0->A1 boundary, vs 2.4us for kp-major order).
                xdt[ei] = inter.tile([P, nKD2, 2, C], F8, tag="xdt",
                                     name=f"xdt{ei}")
                pss = [psp.tile([P, NF], F32, tag="ps", name=f"psa{ei}_{i}")
                       for i in range(2 * nMD)]

                def mm(kp, b):
                    mc, ncc = b // 2, b % 2
                    dmt = dm_t[(ei, kp)]
                    rhs = (dmt[:, ncc] if ei == 0 and kp < 2
                           else dmt[:, :, ncc * NF:(ncc + 1) * NF])
                    nc.tensor.matmul(
                        pss[b][:], x_sb[:, kp, :, mc * P:(mc + 1) * P],
                        rhs,
                        start=(kp == 0), stop=(kp == nKP - 1),
                        perf_mode=DR)

                if not diagonal:
                    for kp in range(nKP):
                        for b in BSEQ:
                            mm(kp, b)
                else:
                    for s in range(nKP + 2 * nMD - 1):
                        for j in range(2 * nMD):
                            kp = s - j
                            if 0 <= kp < nKP:
                                mm(kp, BSEQ[j])
                # PSUM->SBUF casts in BSEQ (= stop) order.  A0's casts
                # alternate vector/scalar — the serial 0.68us/cast DVE
                # chain otherwise trails A1's diagonal ramp by 0.7-2.4us
                # every run (waves 3-6 stall on bank frees).  This only
                # works because the cmt doorbells are emitted AFTER
                # stage_A(0), leaving the scalar engine free at 24-27us.
                # A1's casts stay on vector (B1's deadline is loose and
                # scalar is running B0's Gelu activations by then).
                for idx, b in enumerate(BSEQ):
                    mc, ncc = b // 2, b % 2
                    dst = xdt[ei][:, mc // 2, mc % 2,
                                  ncc * NF:(ncc + 1) * NF]
                    if ei == 0 and idx % 2 == 1:
                        nc.scalar.activation(
                            dst, pss[b][:],
                            mybir.ActivationFunctionType.Copy)
                    else:
                        nc.vector.tensor_copy(dst, pss[b][:])

            ht = {}

            def stage_B(ei):
                # hT[HE, C] = gelu(w18^T xdT + beta), fp8 DR.
                ht[ei] = inter.tile([P, nKH2, 2, C], F8, tag="ht",
                                    name=f"ht{ei}")
                for ncc in range(2):
                    sl = slice(ncc * NF, (ncc + 1) * NF)
                    for mh in range(nMH):
                        ps0 = psp.tile([P, NF], F32, tag="ps")
                        for kd2 in range(nKD2):
                            nc.tensor.matmul(
                                ps0[:],
                                w1_sb[:, ei, kd2, :, mh * P:(mh + 1) * P],
                                xdt[ei][:, kd2, :, sl],
                                start=(kd2 == 0), stop=(kd2 == nKD2 - 1),
                                perf_mode=DR)
                        bia = beta_sb[:, ei * nMH + mh:ei * nMH + mh + 1]
                        nc.scalar.activation(ht[ei][:, mh // 2, mh % 2, sl],
                                             ps0[:], GELU, bias=bia)

            y_tiles = {}

            def stage_C(ei):
                # y[C, O] = hT^T w28, fp8 DR (DoubleRow plane layout for
                # phase D: row c = cp*256 + i*128 + p).
                y_sb = yp.tile([P, nCP, 2, O], F8, tag="y")
                for cc in range(nCC):
                    ps = psp.tile([P, NF], F32, tag="ps")
                    for kh2 in range(nKH2):
                        nc.tensor.matmul(
                            ps[:],
                            ht[ei][:, kh2, :, cc * P:(cc + 1) * P],
                            w2_sb[:, ei, kh2, :, :],
                            start=(kh2 == 0), stop=(kh2 == nKH2 - 1),
                            perf_mode=DR)
                    # split copies across vector/scalar: the serial 8-copy
                    # DVE chain otherwise extends past phase D's start in
                    # the scheduler's timeline and inflates the drain's
                    # semaphore wait targets
                    if cc % 2 == 0:
                        nc.vector.tensor_copy(y_sb[:, cc // 2, cc % 2, :],
                                              ps[:])
                    else:
                        nc.scalar.activation(
                            y_sb[:, cc // 2, cc % 2, :], ps[:],
                            mybir.ActivationFunctionType.Copy)
                y_tiles[ei] = y_sb

            # Stage order: every PSUM-copy / activation dependency gets a
            # full matmul stage of slack before its consumer (A0's copies
            # hide under A1, B0's activations under C0, etc).
            stage_A(0, diagonal=False)
            for kp in range(nCP):
                nc.scalar.dma_start(cmt_t[(0, kp)][:], cmt[0, kp])
            for kp in range(nCP):
                nc.sync.dma_start(cmt_t[(1, kp)][:], cmt[1, kp])
            stage_A(1, diagonal=True)
            stage_B(0)
            stage_C(0)
            stage_B(1)
            stage_C(1)

            # ---- phase D: pout[T, O] = sum_e cmT_e^T y_e (fp8 DR) ----
            for mt in range(nMT):
                ps = psp.tile([P, NF], F32, tag="ps")
                idx = 0
                for ei in range(2):
                    for kp in range(nCP):
                        nc.tensor.matmul(ps[:],
                                         cmt_t[(ei, kp)][:, :, mt * P:(mt + 1) * P],
                                         y_tiles[ei][:, kp, :, :],
                                         start=(idx == 0), stop=(idx == 7),
                                         perf_mode=DR)
                        idx += 1
                ot = outp.tile([P, O], BF16, tag="out")
                # alternate copy engines: vector is busy with C1's y-copies
                # when the drain starts, which otherwise delays it ~5us.
                # Queue routing keeps both HWDGE queues EMPTY when the last
                # chunk's DMAs ring: mt<=12 drains on sync only (74 GB/s
                # demand fits one queue), mt=13/14 go to scalar/sync, and
                # mt=15 is copied in one vector CAST then split across both
                # queues — each 64KB half hits an idle queue.
                if mt == nMT - 1:
                    # final chunk is the end-of-kernel critical chain:
                    # column-split the copy across both PSUM-capable
                    # engines and ship each half from its own (idle) queue
                    # as soon as its copy lands.
                    nc.vector.tensor_copy(ot[:, 0:O // 2], ps[:, 0:O // 2])
                    nc.sync.dma_start(pout[mt * P:(mt + 1) * P, 0:O // 2],
                                      ot[:, 0:O // 2])
                    nc.scalar.activation(ot[:, O // 2:O], ps[:, O // 2:O],
                                         mybir.ActivationFunctionType.Copy)
                    nc.scalar.dma_start(pout[mt * P:(mt + 1) * P, O // 2:O],
                                        ot[:, O // 2:O])
                    continue
                if mt % 2 == 0:
                    nc.scalar.activation(ot[:], ps[:],
                                         mybir.ActivationFunctionType.Copy)
                else:
                    nc.vector.tensor_copy(ot[:], ps[:])
                if mt == nMT - 3:
                    nc.scalar.dma_start(pout[mt * P:(mt + 1) * P, :], ot[:])
                else:
                    nc.sync.dma_start(pout[mt * P:(mt + 1) * P, :], ot[:])

    nc.compile()
    return nc


def get_nc():
    global _NC
    if _NC is None:
        _NC = _build()
    return _NC


def make_in_maps(x, dispatch_mask, combine_array, w1, b1, w2):
    f8 = ml_dtypes.float8_e4m3
    in_maps = []
    # x in partition-major DR plane layout [P, nKP, 2, D] (= x_sb's exact
    # SBUF layout, so ranged pair-loads are clean fat-line DMAs),
    # t = kp*256 + i*128 + p
    x8_by_b = [
        np.ascontiguousarray(
            x[b].reshape(nKP, 2, P, D).transpose(2, 0, 1, 3)).astype(f8)
        for b in range(B)]
    w18 = w1.astype(f8)
    w28 = w2.astype(f8)
    for m in range(8):
        b, g = m // 2, m % 2
        es = slice(2 * g, 2 * g + 2)
        # dm (shifted), t = kp*256 + i*128 + p.  Tile (e0,kp0) ships
        # h-split as dm0 [2(ncc), P, 2(i), 512] (each half one contiguous
        # 128KB DMA — its h0 start-gates the matmul stream); the other 15
        # tiles ship whole as dmr [15, P, 2, C] (256KB entries have ~15%
        # better effective queue throughput than halves).
        dm_f = (np.transpose(dispatch_mask[b, :, es, :], (1, 0, 2))
                - DM_SHIFT).reshape(2, nKP, 2, P, C)
        dm_tile = dm_f.transpose(0, 1, 3, 2, 4)         # [2, nKP, P, 2, C]
        dm0_s = np.ascontiguousarray(
            dm_tile[0, 0:2].reshape(2, P, 2, 2, NF).transpose(0, 3, 1, 2, 4)
        ).astype(f8)                                    # [2, 2, P, 2, NF]
        dmr_s = np.ascontiguousarray(
            np.concatenate([dm_tile[0, 2:], dm_tile[1]], axis=0)
        ).astype(f8)                                    # [14, P, 2, C]
        # cmT -> [2, nCP, P, 2, T], c = cp*256 + i*128 + p
        cmt_s = np.ascontiguousarray(
            np.transpose(combine_array[b, :, es, :], (1, 2, 0))
            .reshape(2, nCP, 2, P, T).transpose(0, 1, 3, 2, 4)).astype(f8)
        # w1 -> [P, 2, nKD2, 2, HE], d = kd2*256 + i*128 + p
        w1_s = np.ascontiguousarray(
            w18[es].reshape(2, nKD2, 2, P, HE).transpose(3, 0, 1, 2, 4))
        # w2 -> [P, 2, nKH2, 2, O], h' = kh2*256 + i*128 + p
        w2_s = np.ascontiguousarray(
            w28[es].reshape(2, nKH2, 2, P, O).transpose(3, 0, 1, 2, 4))
        # beta = w1^T v + b1 in fp32 with EXACT x and w1 (kills the rank-1
        # component of the x / w1 fp8 quantization errors)
        v = DM_SHIFT * x[b].sum(axis=0)                      # [D]
        beta = np.einsum("edh,d->eh", w1[es], v) + b1[es]    # [2, HE]
        beta_s = np.ascontiguousarray(
            beta.reshape(2, nMH, P).transpose(2, 0, 1).reshape(P, 2 * nMH))
        in_maps.append({
            "xb": x8_by_b[b],
            "dm0": dm0_s,
            "dmr": dmr_s,
            "cmt": cmt_s,
            "w1": w1_s,
            "w2": w2_s,
            "beta": beta_s.astype(np.float32),
        })
    return in_maps


def _norm_cdf(z):
    from math import erf
    return 0.5 * (1.0 + np.array([erf(v / np.sqrt(2.0)) for v in z],
                                 dtype=np.float64))


def kernel(x, dispatch_mask, combine_array, w1, b1, w2, b2):
    nc = get_nc()
    x, dispatch_mask, combine_array, w1, b1, w2 = (
        np.asarray(a, dtype=np.float32)
        for a in (x, dispatch_mask, combine_array, w1, b1, w2))
    in_maps = make_in_maps(x, dispatch_mask, combine_array, w1, b1, w2)
    res = bass_utils.run_bass_kernel_spmd(nc, in_maps, core_ids=list(range(8)))
    b2f = np.asarray(b2, dtype=np.float32)
    f8 = ml_dtypes.float8_e4m3
    w1q = w1.astype(f8).astype(np.float32)
    w2q = w2.astype(f8).astype(np.float32)
    ew2 = w2 - w2q                                           # [E, HE, O]
    xq = x.astype(f8).astype(np.float32)
    uq = (dispatch_mask - DM_SHIFT).astype(f8).astype(np.float32)
    out = np.empty((B, T, O), dtype=np.float32)
    for b in range(B):
        p0 = np.asarray(res.results[2 * b]["pout"], dtype=np.float32)
        p1 = np.asarray(res.results[2 * b + 1]["pout"], dtype=np.float32)
        acc = p0 + p1 + b2f
        # w2-quantization rank-1 correction per expert:
        #   out += outer(rowsum_cm, mu_h @ ew2)  with mu_h = E_c[h] estimated
        # analytically: pre-GELU activations are ~N(beta, s2) across c, so
        # mu_h = E[gelu(N(beta, s2))] in closed form (Gaussian integral).
        rs_cm = combine_array[b].sum(axis=2)                 # [T, E]
        v = DM_SHIFT * x[b].sum(axis=0)
        vu = uq[b].var(axis=2)                               # [T, E]
        for e in range(E):
            beta = (w1[e].T @ v + b1[e]).astype(np.float64)  # [HE]
            s2 = (w1q[e] ** 2).T @ ((xq[b] ** 2).T @ vu[:, e])
            s2 = s2.astype(np.float64)
            zr = beta / np.sqrt(1.0 + s2)
            phi = np.exp(-0.5 * zr * zr) / np.sqrt(2.0 * np.pi)
            mu = beta * _norm_cdf(zr) + s2 / np.sqrt(1.0 + s2) * phi
            acc += np.outer(rs_cm[:, e],
                            mu.astype(np.float32) @ ew2[e])
        out[b] = acc
    return out



# revision 49
# speedup vs baseline: 1.0005x; 1.0005x over previous
"""ExpertsChooseMlp Trainium2 kernel — all-fp8 DoubleRow pipeline.

Full inputs in, full output out. Sharding: 8 cores = 4 batches x 2 expert-pairs.
Core m handles batch b=m//2 and experts {2g, 2g+1}, g=m%2. Each core computes
pout[T,O] = sum_{e in pair} combine[b,:,e,:] @ mlp_e(dispatch[b,:,e,:]^T @ x[b]);
the host sums the two partials per batch and adds b2 + rank-1 corrections.

All four matmul phases run as fp8-e4m3 DoubleRow (K=256/pass, ~1.75x bf16):
  xdT[D,C] = matmul(lhsT=x8[T,D],   rhs=u8[T,C],  DR)   (K=T)
  hT[HE,C] = matmul(lhsT=w18[D,HE], rhs=xdT[D,C], DR)   (K=D), GELU + beta
  y[C,O]   = matmul(lhsT=hT[HE,C],  rhs=w28[HE,O],DR)   (K=HE)
  pout[T,O]= matmul(lhsT=cmT[C,T],  rhs=y[C,O],   DR)   (K=C)

fp8 error control (fp32 reference absmax-rel err ~8.7e-3, budget 2e-2):
fp8 quantization errors of per-(t,c) data average out across the capacity
reduction, but errors in operands SHARED across C (x, w1, w2) are correlated
and do not. Three rank-1 corrections cancel those correlated components:
  1. dispatch mask is mean-shifted: u = dm - 0.5 quantized to fp8 (halves the
     mask quant error); the c-independent term v[d] = 0.5*sum_t x[t,d] is
     computed host-side from EXACT fp32 x, killing the rank-1 part of x's
     quant error.
  2. v is folded through fc1 with EXACT fp32 w1: beta = w1^T v + b1 becomes
     the per-partition activation bias, killing the rank-1 part of w1's
     quant error.
  3. w2's correlated error mean_c(h)^T (w2 - w28) is cancelled host-side
     with mu_h = E_c[h] evaluated analytically (pre-GELU activations are
     ~N(beta, s2) across c; E[gelu(N(beta,s2))] has a closed form), adding
     outer(rowsum_cm, mu_h @ ew2) to the output.

Schedule notes (trace-driven; measured 87.2-89.0us HW exec across runs,
vs 128.9us for the bf16 baseline and a ~69.1us pure-matmul floor at the
fp8 peak; dominant fixed costs outside the matmul stream: ~3.9us head
[framework preamble barrier -> first dm tile landed] and ~11.5us tail
[last-DMA completion-sem latency ~1.3us + a compiler-emitted NEFF
epilogue that zeroes all 253 semaphores one instruction each, ~6us,
plus exit barriers/notify]):
  - fp8 DR matmul streams N=512 in ~216ns (the 157 TF/s fp8 peak);
    LDWEIGHTS fully hides under the previous matmul's streaming.
  - All HBM operands are pre-arranged host-side into the exact SBUF
    DoubleRow plane layouts, so every DMA is a big contiguous burst
    (strided gather descriptors measured only ~25-35 GB/s/queue).
    dm carries an extra ncc-half split so each half is one contiguous
    128KB transfer.
  - Only 2 HWDGE queues exist (sync + scalar engines ring their paced
    doorbells, which OCCUPIES the engine until its transfers finish);
    x/dm interleave across both in consumption order — one queue cannot
    sustain phase A's 148 GB/s arrival rate during the DMA ramp (4us of
    A0 stalls when dm-e0 was single-queue).  Entry SIZING beats
    fine-grained JIT: each queue overlaps ~4 in-flight entries and loses
    ~0.3-0.7us per entry to completion-sem gaps, so all-halved dm
    (32 entries) starved the dm-e1 tail (~2us of A1 stalls).  Only dm
    tiles (e0,kp0..1) are h-split — their halves gate the stream start
    and passes 0-1 during the doorbell-limited ramp; everything else
    ships as 256KB entries.  Weights load after the dm streams (B0/C0
    deadlines are late); cmt queues last (needed only by phase D ~52us).
  - Stage order A0,A1,B0,C0,B1,C1,D.  A0 runs kp-major over all 8 PSUM
    banks (dm tiles consumed JIT as they stream); its PSUM->SBUF casts
    alternate vector/scalar in BSEQ bank-stop order (serial vector-only
    casts trailed A1's diagonal ramp by 0.7-2.4us EVERY run; the scalar
    engine is only free for them because the cmt doorbells are emitted
    after stage_A(0) in program order).  A1 runs a (kp+bank)
    diagonal wavefront: bank j's chain starts as soon as A0's cast j
    freed that bank, and wave s needs only dm-e1 tiles 0..s — measured
    0.8-1.5us residual boundary stall vs 2.4us for kp-major (the
    scheduler otherwise reorders A1 bank-major and stalls on the dm-e1
    arrival tail).  Full round-splitting (ncc rounds) was NET WORSE: it
    needs h0-first DMA order, which start-gates the stream (see above).
  - pout is written bf16 (f32's 4MB cannot drain inside phase D on one
    ~125 GB/s write queue; splitting across queues contends DOWN), with
    drain copies alternating scalar/vector (vector is busy with C1's
    y-copies when the drain starts); mt13 ships on the scalar queue and
    the final chunk is column-split vector/scalar so each 64KB half hits
    an idle queue as soon as its copy lands.
  - Warmup: 5 bf16 N=512 matmuls off a vector memset, sized to end at
    the MEDIAN first-dm-half completion (~10.1us; measured 9.7-11.0).
    HAM un-throttles ~3.4-4us after SUSTAINED PE activity starts, and a
    PE idle gap between warmup end and data-ready resets its
    busy-window tracking (HAM then fired 14-17us instead of ~11, i.e.
    6-10 cold 1.2GHz DR matmuls instead of ~2 — measured both ways).
    A 30x N=64 warmup chain measured a chip-wide 5/6 downclock (P0
    power state: every engine 1.2x slower, DMA unaffected, ~+16us) —
    do not use many small-N matmuls here.  gpsimd cannot read PSUM,
    and its SWDGE steals HBM bandwidth if used during the critical dm
    window.  Dummy first-queue entries do NOT absorb the ~1.5us
    doorbell->wire latency (it is per-entry, not queue spin-up).
"""
import sys

sys.path.insert(0, "/opt/trn_rl_repo")

import numpy as np
import ml_dtypes

import concourse.bacc as bacc
import concourse.mybir as mybir
import concourse.tile as tile
from concourse import bass_utils

B, T, D, E, C, HE, O = 4, 2048, 512, 4, 1024, 512, 512
P = 128
nKP = T // (2 * P)   # 8  T pair-chunks (DR: K=256 per matmul)
nMD = D // P         # 4  D-chunks
nMH = HE // P        # 4  HE-chunks
nKD2 = D // (2 * P)  # 2  D pair-chunks
nCC = C // P         # 8  C-chunks
nKH2 = HE // (2 * P) # 2  HE pair-chunks
nMT = T // P         # 16
NF = 512             # matmul free dim (one PSUM bank)
nCP = nCC // 2       # 4  C pair-chunks for combine

F32 = mybir.dt.float32
BF16 = mybir.dt.bfloat16
F8 = mybir.dt.float8e4
GELU = mybir.ActivationFunctionType.Gelu
DR = mybir.MatmulPerfMode.DoubleRow
DM_SHIFT = 0.5

_NC = None


def _build():
    nc = bacc.Bacc("TRN2", target_bir_lowering=False, debug=False,
                   enable_asserts=False, num_devices=1)
    # All inputs pre-arranged host-side into SBUF plane layouts (contiguous).
    # dm tiles (e0,kp0..1) ship h-split via dm0 [2(tile), 2(ncc), P, 2, NF]
    # (each 128KB half one contiguous DMA — they gate the stream start and
    # the first kp passes); the other 14 tiles ship whole via dmr (256KB
    # entries have ~15% better effective queue throughput than halves).
    # x is partition-major = x_sb's exact SBUF layout.
    xb = nc.dram_tensor("xb", [P, nKP, 2, D], F8, kind="ExternalInput").ap()
    dm0 = nc.dram_tensor("dm0", [3, 2, P, 2, NF], F8, kind="ExternalInput").ap()
    dmr = nc.dram_tensor("dmr", [2 * nKP - 3, P, 2, C], F8,
                         kind="ExternalInput").ap()
    cmt = nc.dram_tensor("cmt", [2, nCP, P, 2, T], F8, kind="ExternalInput").ap()
    w1 = nc.dram_tensor("w1", [P, 2, nKD2, 2, HE], F8, kind="ExternalInput").ap()
    w2 = nc.dram_tensor("w2", [P, 2, nKH2, 2, O], F8, kind="ExternalInput").ap()
    beta = nc.dram_tensor("beta", [P, 2 * nMH], F32, kind="ExternalInput").ap()
    # pout in bf16: one HWDGE queue writes ~125 GB/s (splitting across queues
    # contends DOWN to ~105), so f32's 4MB can't drain inside phase D's 27us
    # — bf16's 2MB can. Costs ~+3e-3 absmax-rel worst case.
    pout = nc.dram_tensor("pout", [T, O], BF16, kind="ExternalOutput").ap()

    with tile.TileContext(nc) as tc:
        with (
            tc.tile_pool(name="const", bufs=1) as const,
            tc.tile_pool(name="dmp", bufs=16) as dmp,
            tc.tile_pool(name="cmp", bufs=8) as cmp_,
            tc.tile_pool(name="inter", bufs=2) as inter,
            tc.tile_pool(name="yp", bufs=2) as yp,
            tc.tile_pool(name="outp", bufs=2) as outp,
            tc.tile_pool(name="psum", bufs=8, space="PSUM") as psp,
        ):
            # ---- DMA plan (2 HWDGE queues, deadline-ordered) ----
            # Entry sizing matters more than fine-grained JIT: each queue
            # overlaps ~4 in-flight entries and loses ~0.3-0.7us per entry
            # to completion gaps, so 32 half-tile dm entries measured ~15%
            # less effective throughput than 256KB entries — which starved
            # the dm-e1 tail right when A1's diagonal needed it.  Only
            # tile (e0,kp0) is h-split (its h0 half start-gates the whole
            # matmul stream); everything else ships as big contiguous
            # entries, deadline-ordered and alternating across queues.
            x_sb = const.tile([P, nKP, 2, D], F8)
            dm_t = {}
            for kp in range(3):
                dm_t[(0, kp)] = dmp.tile([P, 2, 2, NF], F8, tag="dm",
                                         name=f"dm_t0{kp}s")
            for j in range(2 * nKP - 3):
                ei, kp = (0, j + 3) if j < nKP - 3 else (1, j - (nKP - 3))
                dm_t[(ei, kp)] = dmp.tile([P, 2, C], F8, tag="dm",
                                          name=f"dm_t{ei}{kp}")
            nc.sync.dma_start(dm_t[(0, 0)][:, 0], dm0[0, 0])
            nc.scalar.dma_start(x_sb[:, 0, :, :], xb[:, 0])
            nc.sync.dma_start(dm_t[(0, 0)][:, 1], dm0[0, 1])
            nc.scalar.dma_start(x_sb[:, 1, :, :], xb[:, 1])
            nc.sync.dma_start(dm_t[(0, 1)][:, 1], dm0[1, 1])
            nc.scalar.dma_start(dm_t[(0, 1)][:, 0], dm0[1, 0])
            nc.sync.dma_start(dm_t[(0, 2)][:, 0], dm0[2, 0])
            nc.scalar.dma_start(dm_t[(0, 2)][:, 1], dm0[2, 1])
            nc.scalar.dma_start(x_sb[:, 2, :, :], xb[:, 2])
            nc.sync.dma_start(x_sb[:, 3, :, :], xb[:, 3])
            nc.scalar.dma_start(dm_t[(0, 3)][:], dmr[0])
            nc.sync.dma_start(dm_t[(0, 4)][:], dmr[1])
            nc.scalar.dma_start(x_sb[:, 4, :, :], xb[:, 4])
            nc.sync.dma_start(x_sb[:, 5, :, :], xb[:, 5])
            nc.scalar.dma_start(dm_t[(0, 5)][:], dmr[2])
            nc.sync.dma_start(dm_t[(0, 6)][:], dmr[3])
            nc.scalar.dma_start(x_sb[:, 6, :, :], xb[:, 6])
            nc.sync.dma_start(x_sb[:, 7, :, :], xb[:, 7])
            nc.scalar.dma_start(dm_t[(0, 7)][:], dmr[4])
            for kp in range(nKP):
                eng = nc.scalar if kp % 2 == 0 else nc.sync
                eng.dma_start(dm_t[(1, kp)][:], dmr[nKP - 3 + kp])
            w1_sb = const.tile([P, 2, nKD2, 2, HE], F8)
            nc.sync.dma_start(w1_sb[:], w1[:])
            beta_sb = const.tile([P, 2 * nMH], F32)
            nc.sync.dma_start(beta_sb[:], beta[:])
            w2_sb = const.tile([P, 2, nKH2, 2, O], F8)
            nc.scalar.dma_start(w2_sb[:], w2[:])
            cmt_t = {}
            for ei in range(2):
                for kp in range(nCP):
                    t_ = cmp_.tile([P, 2, T], F8, tag="cmt")
                    cmt_t[(ei, kp)] = t_
            # cmt dma_starts are emitted AFTER stage_A(0): their doorbells
            # would otherwise sit ahead of A0's scalar-side casts in the
            # scalar engine's stream and push them past 30us (doorbells
            # occupy the engine until flow control clears).  cmt has ~12us
            # of slack (needed ~52us, lands ~41us even when issued there).

            # ---- HAM warmup: 5 bf16 matmuls on a vector-memset tile during
            # the initial DMA wait.  Sized to end ~10.1us = near the MEDIAN
            # first-dm-half completion (measured 9.7-11.0): a PE idle gap
            # between warmup end and data-ready resets HAM's busy-window
            # tracking (HAM then fired at 14-17us instead of ~11.5, i.e.
            # 6-10 cold DR matmuls instead of ~2).  (A 30x N=64 chain
            # measured a chip-wide 2.0 GHz power-state downclock — do not
            # use many small-N matmuls here.)
            warm = const.tile([P, NF], BF16)
            nc.vector.memset(warm[:], 0.0)
            ps_w = psp.tile([P, NF], F32, tag="ps", name="ps_warm")
            for i in range(5):
                nc.tensor.matmul(ps_w[:], warm[:, 0:P], warm[:],
                                 start=(i == 0), stop=(i == 4))

            xdt = {}
            # bank b = 2*mc + ncc; BSEQ = bank completion order of the
            # ncc-major kp pass (= CAST emission order = the next phase's
            # bank-free order).
            BSEQ = [0, 2, 4, 6, 1, 3, 5, 7]

            def stage_A(ei, diagonal):
                # xdT[D, C] = x8^T u8, fp8 DR, all 8 PSUM banks.
                # A0 runs kp-major: each dm tile is consumed right as it
                # streams in (DMA-JIT at the ramp).  A1 runs a (kp+bank)
                # diagonal wavefront: bank j's chain starts as soon as A0's
                # CAST j freed that PSUM bank, and wave s only needs dm-e1
                # tiles 0..s — so neither the CAST cadence nor the dm-e1
                # arrival tail stalls the PE (measured 0.8us residual at
                # the A0->A1 boundary, vs 2.4us for kp-major order).
                xdt[ei] = inter.tile([P, nKD2, 2, C], F8, tag="xdt",
                                     name=f"xdt{ei}")
                pss = [psp.tile([P, NF], F32, tag="ps", name=f"psa{ei}_{i}")
                       for i in range(2 * nMD)]

                def mm(kp, b):
                    mc, ncc = b // 2, b % 2
                    dmt = dm_t[(ei, kp)]
                    rhs = (dmt[:, ncc] if ei == 0 and kp < 3
                           else dmt[:, :, ncc * NF:(ncc + 1) * NF])
                    nc.tensor.matmul(
                        pss[b][:], x_sb[:, kp, :, mc * P:(mc + 1) * P],
                        rhs,
                        start=(kp == 0), stop=(kp == nKP - 1),
                        perf_mode=DR)

                if not diagonal:
                    for kp in range(nKP):
                        for b in BSEQ:
                            mm(kp, b)
                else:
                    for s in range(nKP + 2 * nMD - 1):
                        for j in range(2 * nMD):
                            kp = s - j
                            if 0 <= kp < nKP:
                                mm(kp, BSEQ[j])
                # PSUM->SBUF casts in BSEQ (= stop) order.  A0's casts
                # alternate vector/scalar — the serial 0.68us/cast DVE
                # chain otherwise trails A1's diagonal ramp by 0.7-2.4us
                # every run (waves 3-6 stall on bank frees).  This only
                # works because the cmt doorbells are emitted AFTER
                # stage_A(0), leaving the scalar engine free at 24-27us.
                # A1's casts stay on vector (B1's deadline is loose and
                # scalar is running B0's Gelu activations by then).
                for idx, b in enumerate(BSEQ):
                    mc, ncc = b // 2, b % 2
                    dst = xdt[ei][:, mc // 2, mc % 2,
                                  ncc * NF:(ncc + 1) * NF]
                    if ei == 0 and idx % 2 == 1:
                        nc.scalar.activation(
                            dst, pss[b][:],
                            mybir.ActivationFunctionType.Copy)
                    else:
                        nc.vector.tensor_copy(dst, pss[b][:])

            ht = {}

            def stage_B(ei):
                # hT[HE, C] = gelu(w18^T xdT + beta), fp8 DR.
                ht[ei] = inter.tile([P, nKH2, 2, C], F8, tag="ht",
                                    name=f"ht{ei}")
                for ncc in range(2):
                    sl = slice(ncc * NF, (ncc + 1) * NF)
                    for mh in range(nMH):
                        ps0 = psp.tile([P, NF], F32, tag="ps")
                        for kd2 in range(nKD2):
                            nc.tensor.matmul(
                                ps0[:],
                                w1_sb[:, ei, kd2, :, mh * P:(mh + 1) * P],
                                xdt[ei][:, kd2, :, sl],
                                start=(kd2 == 0), stop=(kd2 == nKD2 - 1),
                                perf_mode=DR)
                        bia = beta_sb[:, ei * nMH + mh:ei * nMH + mh + 1]
                        nc.scalar.activation(ht[ei][:, mh // 2, mh % 2, sl],
                                             ps0[:], GELU, bias=bia)

            y_tiles = {}

            def stage_C(ei):
                # y[C, O] = hT^T w28, fp8 DR (DoubleRow plane layout for
                # phase D: row c = cp*256 + i*128 + p).
                y_sb = yp.tile([P, nCP, 2, O], F8, tag="y")
                for cc in range(nCC):
                    ps = psp.tile([P, NF], F32, tag="ps")
                    for kh2 in range(nKH2):
                        nc.tensor.matmul(
                            ps[:],
                            ht[ei][:, kh2, :, cc * P:(cc + 1) * P],
                            w2_sb[:, ei, kh2, :, :],
                            start=(kh2 == 0), stop=(kh2 == nKH2 - 1),
                            perf_mode=DR)
                    # split copies across vector/scalar: the serial 8-copy
                    # DVE chain otherwise extends past phase D's start in
                    # the scheduler's timeline and inflates the drain's
                    # semaphore wait targets
                    if cc % 2 == 0:
                        nc.vector.tensor_copy(y_sb[:, cc // 2, cc % 2, :],
                                              ps[:])
                    else:
                        nc.scalar.activation(
                            y_sb[:, cc // 2, cc % 2, :], ps[:],
                            mybir.ActivationFunctionType.Copy)
                y_tiles[ei] = y_sb

            # Stage order: every PSUM-copy / activation dependency gets a
            # full matmul stage of slack before its consumer (A0's copies
            # hide under A1, B0's activations under C0, etc).
            stage_A(0, diagonal=False)
            for kp in range(nCP):
                nc.scalar.dma_start(cmt_t[(0, kp)][:], cmt[0, kp])
            for kp in range(nCP):
                nc.sync.dma_start(cmt_t[(1, kp)][:], cmt[1, kp])
            stage_A(1, diagonal=True)
            stage_B(0)
            stage_C(0)
            stage_B(1)
            stage_C(1)

            # ---- phase D: pout[T, O] = sum_e cmT_e^T y_e (fp8 DR) ----
            for mt in range(nMT):
                ps = psp.tile([P, NF], F32, tag="ps")
                idx = 0
                for ei in range(2):
                    for kp in range(nCP):
                        nc.tensor.matmul(ps[:],
                                         cmt_t[(ei, kp)][:, :, mt * P:(mt + 1) * P],
                                         y_tiles[ei][:, kp, :, :],
                                         start=(idx == 0), stop=(idx == 7),
                                         perf_mode=DR)
                        idx += 1
                ot = outp.tile([P, O], BF16, tag="out")
                # alternate copy engines: vector is busy with C1's y-copies
                # when the drain starts, which otherwise delays it ~5us.
                # Queue routing keeps both HWDGE queues EMPTY when the last
                # chunk's DMAs ring: mt<=12 drains on sync only (74 GB/s
                # demand fits one queue), mt=13/14 go to scalar/sync, and
                # mt=15 is copied in one vector CAST then split across both
                # queues — each 64KB half hits an idle queue.
                if mt == nMT - 1:
                    # final chunk is the end-of-kernel critical chain:
                    # column-split the copy across both PSUM-capable
                    # engines and ship each half from its own (idle) queue
                    # as soon as its copy lands.
                    nc.vector.tensor_copy(ot[:, 0:O // 2], ps[:, 0:O // 2])
                    nc.sync.dma_start(pout[mt * P:(mt + 1) * P, 0:O // 2],
                                      ot[:, 0:O // 2])
                    nc.scalar.activation(ot[:, O // 2:O], ps[:, O // 2:O],
                                         mybir.ActivationFunctionType.Copy)
                    nc.scalar.dma_start(pout[mt * P:(mt + 1) * P, O // 2:O],
                                        ot[:, O // 2:O])
                    continue
                if mt % 2 == 0:
                    nc.scalar.activation(ot[:], ps[:],
                                         mybir.ActivationFunctionType.Copy)
                else:
                    nc.vector.tensor_copy(ot[:], ps[:])
                if mt == nMT - 3:
                    nc.scalar.dma_start(pout[mt * P:(mt + 1) * P, :], ot[:])
                else:
                    nc.sync.dma_start(pout[mt * P:(mt + 1) * P, :], ot[:])

    nc.compile()
    return nc


def get_nc():
    global _NC
    if _NC is None:
        _NC = _build()
    return _NC


def make_in_maps(x, dispatch_mask, combine_array, w1, b1, w2):
    f8 = ml_dtypes.float8_e4m3
    in_maps = []
    # x in partition-major DR plane layout [P, nKP, 2, D] (= x_sb's exact
    # SBUF layout, so ranged pair-loads are clean fat-line DMAs),
    # t = kp*256 + i*128 + p
    x8_by_b = [
        np.ascontiguousarray(
            x[b].reshape(nKP, 2, P, D).transpose(2, 0, 1, 3)).astype(f8)
        for b in range(B)]
    w18 = w1.astype(f8)
    w28 = w2.astype(f8)
    for m in range(8):
        b, g = m // 2, m % 2
        es = slice(2 * g, 2 * g + 2)
        # dm (shifted), t = kp*256 + i*128 + p.  Tile (e0,kp0) ships
        # h-split as dm0 [2(ncc), P, 2(i), 512] (each half one contiguous
        # 128KB DMA — its h0 start-gates the matmul stream); the other 15
        # tiles ship whole as dmr [15, P, 2, C] (256KB entries have ~15%
        # better effective queue throughput than halves).
        dm_f = (np.transpose(dispatch_mask[b, :, es, :], (1, 0, 2))
                - DM_SHIFT).reshape(2, nKP, 2, P, C)
        dm_tile = dm_f.transpose(0, 1, 3, 2, 4)         # [2, nKP, P, 2, C]
        dm0_s = np.ascontiguousarray(
            dm_tile[0, 0:3].reshape(3, P, 2, 2, NF).transpose(0, 3, 1, 2, 4)
        ).astype(f8)                                    # [3, 2, P, 2, NF]
        dmr_s = np.ascontiguousarray(
            np.concatenate([dm_tile[0, 3:], dm_tile[1]], axis=0)
        ).astype(f8)                                    # [13, P, 2, C]
        # cmT -> [2, nCP, P, 2, T], c = cp*256 + i*128 + p
        cmt_s = np.ascontiguousarray(
            np.transpose(combine_array[b, :, es, :], (1, 2, 0))
            .reshape(2, nCP, 2, P, T).transpose(0, 1, 3, 2, 4)).astype(f8)
        # w1 -> [P, 2, nKD2, 2, HE], d = kd2*256 + i*128 + p
        w1_s = np.ascontiguousarray(
            w18[es].reshape(2, nKD2, 2, P, HE).transpose(3, 0, 1, 2, 4))
        # w2 -> [P, 2, nKH2, 2, O], h' = kh2*256 + i*128 + p
        w2_s = np.ascontiguousarray(
            w28[es].reshape(2, nKH2, 2, P, O).transpose(3, 0, 1, 2, 4))
        # beta = w1^T v + b1 in fp32 with EXACT x and w1 (kills the rank-1
        # component of the x / w1 fp8 quantization errors)
        v = DM_SHIFT * x[b].sum(axis=0)                      # [D]
        beta = np.einsum("edh,d->eh", w1[es], v) + b1[es]    # [2, HE]
        beta_s = np.ascontiguousarray(
            beta.reshape(2, nMH, P).transpose(2, 0, 1).reshape(P, 2 * nMH))
        in_maps.append({
            "xb": x8_by_b[b],
            "dm0": dm0_s,
            "dmr": dmr_s,
            "cmt": cmt_s,
            "w1": w1_s,
            "w2": w2_s,
            "beta": beta_s.astype(np.float32),
        })
    return in_maps


def _norm_cdf(z):
    from math import erf
    return 0.5 * (1.0 + np.array([erf(v / np.sqrt(2.0)) for v in z],
                                 dtype=np.float64))


def kernel(x, dispatch_mask, combine_array, w1, b1, w2, b2):
    nc = get_nc()
    x, dispatch_mask, combine_array, w1, b1, w2 = (
        np.asarray(a, dtype=np.float32)
        for a in (x, dispatch_mask, combine_array, w1, b1, w2))
    in_maps = make_in_maps(x, dispatch_mask, combine_array, w1, b1, w2)
    res = bass_utils.run_bass_kernel_spmd(nc, in_maps, core_ids=list(range(8)))
    b2f = np.asarray(b2, dtype=np.float32)
    f8 = ml_dtypes.float8_e4m3
    w1q = w1.astype(f8).astype(np.float32)
    w2q = w2.astype(f8).astype(np.float32)
    ew2 = w2 - w2q                                           # [E, HE, O]
    xq = x.astype(f8).astype(np.float32)
    uq = (dispatch_mask - DM_SHIFT).astype(f8).astype(np.float32)
    out = np.empty((B, T, O), dtype=np.float32)
    for b in range(B):
        p0 = np.asarray(res.results[2 * b]["pout"], dtype=np.float32)
        p1 = np.asarray(res.results[2 * b + 1]["pout"], dtype=np.float32)
        acc = p0 + p1 + b2f
        # w2-quantization rank-1 correction per expert:
        #   out += outer(rowsum_cm, mu_h @ ew2)  with mu_h = E_c[h] estimated
        # analytically: pre-GELU activations are ~N(beta, s2) across c, so
        # mu_h = E[gelu(N(beta, s2))] in closed form (Gaussian integral).
        rs_cm = combine_array[b].sum(axis=2)                 # [T, E]
        v = DM_SHIFT * x[b].sum(axis=0)
        vu = uq[b].var(axis=2)                               # [T, E]
        for e in range(E):
            beta = (w1[e].T @ v + b1[e]).astype(np.float64)  # [HE]
            s2 = (w1q[e] ** 2).T @ ((xq[b] ** 2).T @ vu[:, e])
            s2 = s2.astype(np.float64)
            zr = beta / np.sqrt(1.0 + s2)
            phi = np.exp(-0.5 * zr * zr) / np.sqrt(2.0 * np.pi)
            mu = beta * _norm_cdf(zr) + s2 / np.sqrt(1.0 + s2) * phi
            acc += np.outer(rs_cm[:, e],
                            mu.astype(np.float32) @ ew2[e])
        out[b] = acc
    return out



# revision 51
# speedup vs baseline: 1.0018x; 1.0013x over previous
"""ExpertsChooseMlp Trainium2 kernel — all-fp8 DoubleRow pipeline.

Full inputs in, full output out. Sharding: 8 cores = 4 batches x 2 expert-pairs.
Core m handles batch b=m//2 and experts {2g, 2g+1}, g=m%2. Each core computes
pout[T,O] = sum_{e in pair} combine[b,:,e,:] @ mlp_e(dispatch[b,:,e,:]^T @ x[b]);
the host sums the two partials per batch and adds b2 + rank-1 corrections.

All four matmul phases run as fp8-e4m3 DoubleRow (K=256/pass, ~1.75x bf16):
  xdT[D,C] = matmul(lhsT=x8[T,D],   rhs=u8[T,C],  DR)   (K=T)
  hT[HE,C] = matmul(lhsT=w18[D,HE], rhs=xdT[D,C], DR)   (K=D), GELU + beta
  y[C,O]   = matmul(lhsT=hT[HE,C],  rhs=w28[HE,O],DR)   (K=HE)
  pout[T,O]= matmul(lhsT=cmT[C,T],  rhs=y[C,O],   DR)   (K=C)

fp8 error control (fp32 reference absmax-rel err ~8.7e-3, budget 2e-2):
fp8 quantization errors of per-(t,c) data average out across the capacity
reduction, but errors in operands SHARED across C (x, w1, w2) are correlated
and do not. Three rank-1 corrections cancel those correlated components:
  1. dispatch mask is mean-shifted: u = dm - 0.5 quantized to fp8 (halves the
     mask quant error); the c-independent term v[d] = 0.5*sum_t x[t,d] is
     computed host-side from EXACT fp32 x, killing the rank-1 part of x's
     quant error.
  2. v is folded through fc1 with EXACT fp32 w1: beta = w1^T v + b1 becomes
     the per-partition activation bias, killing the rank-1 part of w1's
     quant error.
  3. w2's correlated error mean_c(h)^T (w2 - w28) is cancelled host-side
     with mu_h = E_c[h] evaluated analytically (pre-GELU activations are
     ~N(beta, s2) across c; E[gelu(N(beta,s2))] has a closed form), adding
     outer(rowsum_cm, mu_h @ ew2) to the output.

Schedule notes (trace-driven; measured 87.2-89.0us HW exec across runs,
vs 128.9us for the bf16 baseline and a ~69.1us pure-matmul floor at the
fp8 peak; dominant fixed costs outside the matmul stream: ~3.9us head
[framework preamble barrier -> first dm tile landed] and ~11.5us tail
[last-DMA completion-sem latency ~1.3us + a compiler-emitted NEFF
epilogue that zeroes all 253 semaphores one instruction each, ~6us,
plus exit barriers/notify]):
  - fp8 DR matmul streams N=512 in ~216ns (the 157 TF/s fp8 peak);
    LDWEIGHTS fully hides under the previous matmul's streaming.
  - All HBM operands are pre-arranged host-side into the exact SBUF
    DoubleRow plane layouts, so every DMA is a big contiguous burst
    (strided gather descriptors measured only ~25-35 GB/s/queue).
    dm carries an extra ncc-half split so each half is one contiguous
    128KB transfer.
  - Only 2 HWDGE queues exist (sync + scalar engines ring their paced
    doorbells, which OCCUPIES the engine until its transfers finish);
    x/dm interleave across both in consumption order — one queue cannot
    sustain phase A's 148 GB/s arrival rate during the DMA ramp (4us of
    A0 stalls when dm-e0 was single-queue).  Entry SIZING beats
    fine-grained JIT: each queue overlaps ~4 in-flight entries and loses
    ~0.3-0.7us per entry to completion-sem gaps, so all-halved dm
    (32 entries) starved the dm-e1 tail (~2us of A1 stalls).  Only dm
    tiles (e0,kp0..2) are h-split — their halves gate the stream start
    and passes 0-2 during the doorbell-limited ramp; everything else
    ships as 256KB entries.  Weights load after the dm streams (B0/C0
    deadlines are late); cmt queues last (needed only by phase D ~52us).
  - Stage order A0,A1,B0,C0,B1,C1,D.  A0 runs kp-major over all 8 PSUM
    banks (dm tiles consumed JIT as they stream); its PSUM->SBUF casts
    alternate vector/scalar in BSEQ bank-stop order (serial vector-only
    casts trailed A1's diagonal ramp by 0.7-2.4us EVERY run; the scalar
    engine is only free for them because the cmt doorbells are emitted
    after stage_A(0) in program order).  A1 runs a (kp+bank)
    diagonal wavefront: bank j's chain starts as soon as A0's cast j
    freed that bank, and wave s needs only dm-e1 tiles 0..s — measured
    0.8-1.5us residual boundary stall vs 2.4us for kp-major (the
    scheduler otherwise reorders A1 bank-major and stalls on the dm-e1
    arrival tail).  Full round-splitting (ncc rounds) was NET WORSE: it
    needs h0-first DMA order, which start-gates the stream (see above).
  - pout is written bf16 (f32's 4MB cannot drain inside phase D on one
    ~125 GB/s write queue; splitting across queues contends DOWN), with
    drain copies alternating scalar/vector (vector is busy with C1's
    y-copies when the drain starts); mt13 ships on the scalar queue and
    the final chunk is column-split vector/scalar so each 64KB half hits
    an idle queue as soon as its copy lands.
  - Warmup: 5 bf16 N=512 matmuls off a vector memset, sized to end at
    the MEDIAN first-dm-half completion (~10.1us; measured 9.7-11.0).
    HAM un-throttles ~3.4-4us after SUSTAINED PE activity starts, and a
    PE idle gap between warmup end and data-ready resets its
    busy-window tracking (HAM then fired 14-17us instead of ~11, i.e.
    6-10 cold 1.2GHz DR matmuls instead of ~2 — measured both ways).
    A 30x N=64 warmup chain measured a chip-wide 5/6 downclock (P0
    power state: every engine 1.2x slower, DMA unaffected, ~+16us) —
    do not use many small-N matmuls here.  gpsimd cannot read PSUM,
    and its SWDGE steals HBM bandwidth if used during the critical dm
    window.  Dummy first-queue entries do NOT absorb the ~1.5us
    doorbell->wire latency (it is per-entry, not queue spin-up).
"""
import sys

sys.path.insert(0, "/opt/trn_rl_repo")

import numpy as np
import ml_dtypes

import concourse.bacc as bacc
import concourse.mybir as mybir
import concourse.tile as tile
from concourse import bass_utils

B, T, D, E, C, HE, O = 4, 2048, 512, 4, 1024, 512, 512
P = 128
nKP = T // (2 * P)   # 8  T pair-chunks (DR: K=256 per matmul)
nMD = D // P         # 4  D-chunks
nMH = HE // P        # 4  HE-chunks
nKD2 = D // (2 * P)  # 2  D pair-chunks
nCC = C // P         # 8  C-chunks
nKH2 = HE // (2 * P) # 2  HE pair-chunks
nMT = T // P         # 16
NF = 512             # matmul free dim (one PSUM bank)
nCP = nCC // 2       # 4  C pair-chunks for combine

F32 = mybir.dt.float32
BF16 = mybir.dt.bfloat16
F8 = mybir.dt.float8e4
GELU = mybir.ActivationFunctionType.Gelu
DR = mybir.MatmulPerfMode.DoubleRow
DM_SHIFT = 0.5

_NC = None


def _build():
    nc = bacc.Bacc("TRN2", target_bir_lowering=False, debug=False,
                   enable_asserts=False, num_devices=1)
    # All inputs pre-arranged host-side into SBUF plane layouts (contiguous).
    # dm tiles (e0,kp0..2) ship h-split via dm0 [3(tile), 2(ncc), P, 2, NF]
    # (each 128KB half one contiguous DMA — they gate the stream start and
    # the first kp passes, whose byte demand sits at the early aggregate
    # bandwidth limit); the other 13 tiles ship whole via dmr (256KB
    # entries have ~15% better effective queue throughput than halves).
    # x is partition-major = x_sb's exact SBUF layout.
    xb = nc.dram_tensor("xb", [P, nKP, 2, D], F8, kind="ExternalInput").ap()
    dm0 = nc.dram_tensor("dm0", [3, 2, P, 2, NF], F8, kind="ExternalInput").ap()
    dmr = nc.dram_tensor("dmr", [2 * nKP - 3, P, 2, C], F8,
                         kind="ExternalInput").ap()
    cmt = nc.dram_tensor("cmt", [2, nCP, P, 2, T], F8, kind="ExternalInput").ap()
    w1 = nc.dram_tensor("w1", [P, 2, nKD2, 2, HE], F8, kind="ExternalInput").ap()
    w2 = nc.dram_tensor("w2", [P, 2, nKH2, 2, O], F8, kind="ExternalInput").ap()
    beta = nc.dram_tensor("beta", [P, 2 * nMH], F32, kind="ExternalInput").ap()
    # pout in bf16: one HWDGE queue writes ~125 GB/s (splitting across queues
    # contends DOWN to ~105), so f32's 4MB can't drain inside phase D's 27us
    # — bf16's 2MB can. Costs ~+3e-3 absmax-rel worst case.
    pout = nc.dram_tensor("pout", [T, O], BF16, kind="ExternalOutput").ap()

    with tile.TileContext(nc) as tc:
        with (
            tc.tile_pool(name="const", bufs=1) as const,
            tc.tile_pool(name="dmp", bufs=16) as dmp,
            tc.tile_pool(name="cmp", bufs=8) as cmp_,
            tc.tile_pool(name="inter", bufs=2) as inter,
            tc.tile_pool(name="yp", bufs=2) as yp,
            tc.tile_pool(name="outp", bufs=2) as outp,
            tc.tile_pool(name="psum", bufs=8, space="PSUM") as psp,
        ):
            # ---- DMA plan (2 HWDGE queues, deadline-ordered) ----
            # Entry sizing matters more than fine-grained JIT: each queue
            # overlaps ~4 in-flight entries and loses ~0.3-0.7us per entry
            # to completion gaps, so 32 half-tile dm entries measured ~15%
            # less effective throughput than 256KB entries — which starved
            # the dm-e1 tail right when A1's diagonal needed it.  Only
            # tile (e0,kp0) is h-split (its h0 half start-gates the whole
            # matmul stream); everything else ships as big contiguous
            # entries, deadline-ordered and alternating across queues.
            x_sb = const.tile([P, nKP, 2, D], F8)
            dm_t = {}
            for kp in range(3):
                dm_t[(0, kp)] = dmp.tile([P, 2, 2, NF], F8, tag="dm",
                                         name=f"dm_t0{kp}s")
            for j in range(2 * nKP - 3):
                ei, kp = (0, j + 3) if j < nKP - 3 else (1, j - (nKP - 3))
                dm_t[(ei, kp)] = dmp.tile([P, 2, C], F8, tag="dm",
                                          name=f"dm_t{ei}{kp}")
            nc.sync.dma_start(dm_t[(0, 0)][:, 0], dm0[0, 0])
            nc.scalar.dma_start(x_sb[:, 0, :, :], xb[:, 0])
            nc.sync.dma_start(dm_t[(0, 0)][:, 1], dm0[0, 1])
            nc.scalar.dma_start(x_sb[:, 1, :, :], xb[:, 1])
            nc.sync.dma_start(dm_t[(0, 1)][:, 1], dm0[1, 1])
            nc.scalar.dma_start(dm_t[(0, 1)][:, 0], dm0[1, 0])
            nc.sync.dma_start(dm_t[(0, 2)][:, 0], dm0[2, 0])
            nc.scalar.dma_start(dm_t[(0, 2)][:, 1], dm0[2, 1])
            nc.scalar.dma_start(x_sb[:, 2, :, :], xb[:, 2])
            nc.sync.dma_start(x_sb[:, 3, :, :], xb[:, 3])
            nc.scalar.dma_start(dm_t[(0, 3)][:], dmr[0])
            nc.sync.dma_start(dm_t[(0, 4)][:], dmr[1])
            nc.scalar.dma_start(x_sb[:, 4, :, :], xb[:, 4])
            nc.sync.dma_start(x_sb[:, 5, :, :], xb[:, 5])
            nc.scalar.dma_start(dm_t[(0, 5)][:], dmr[2])
            nc.sync.dma_start(dm_t[(0, 6)][:], dmr[3])
            nc.scalar.dma_start(x_sb[:, 6, :, :], xb[:, 6])
            nc.sync.dma_start(x_sb[:, 7, :, :], xb[:, 7])
            nc.scalar.dma_start(dm_t[(0, 7)][:], dmr[4])
            for kp in range(nKP):
                eng = nc.scalar if kp % 2 == 0 else nc.sync
                eng.dma_start(dm_t[(1, kp)][:], dmr[nKP - 3 + kp])
            w1_sb = const.tile([P, 2, nKD2, 2, HE], F8)
            nc.sync.dma_start(w1_sb[:], w1[:])
            beta_sb = const.tile([P, 2 * nMH], F32)
            nc.sync.dma_start(beta_sb[:], beta[:])
            w2_sb = const.tile([P, 2, nKH2, 2, O], F8)
            nc.scalar.dma_start(w2_sb[:], w2[:])
            cmt_t = {}
            for ei in range(2):
                for kp in range(nCP):
                    t_ = cmp_.tile([P, 2, T], F8, tag="cmt")
                    cmt_t[(ei, kp)] = t_
            # cmt dma_starts are emitted AFTER stage_A(0): their doorbells
            # would otherwise sit ahead of A0's scalar-side casts in the
            # scalar engine's stream and push them past 30us (doorbells
            # occupy the engine until flow control clears).  cmt has ~12us
            # of slack (needed ~52us, lands ~41us even when issued there).

            # ---- HAM warmup: 5 bf16 matmuls on a vector-memset tile during
            # the initial DMA wait.  Sized to end ~10.1us = near the MEDIAN
            # first-dm-half completion (measured 9.7-11.0): a PE idle gap
            # between warmup end and data-ready resets HAM's busy-window
            # tracking (HAM then fired at 14-17us instead of ~11.5, i.e.
            # 6-10 cold DR matmuls instead of ~2).  (A 30x N=64 chain
            # measured a chip-wide 2.0 GHz power-state downclock — do not
            # use many small-N matmuls here.)
            warm = const.tile([P, NF], BF16)
            nc.vector.memset(warm[:], 0.0)
            ps_w = psp.tile([P, NF], F32, tag="ps", name="ps_warm")
            for i in range(5):
                nc.tensor.matmul(ps_w[:], warm[:, 0:P], warm[:],
                                 start=(i == 0), stop=(i == 4))

            xdt = {}
            # bank b = 2*mc + ncc; BSEQ = bank completion order of the
            # ncc-major kp pass (= CAST emission order = the next phase's
            # bank-free order).
            BSEQ = [0, 2, 4, 6, 1, 3, 5, 7]

            def stage_A(ei, diagonal):
                # xdT[D, C] = x8^T u8, fp8 DR, all 8 PSUM banks.
                # A0 runs kp-major: each dm tile is consumed right as it
                # streams in (DMA-JIT at the ramp).  A1 runs a (kp+bank)
                # diagonal wavefront: bank j's chain starts as soon as A0's
                # CAST j freed that PSUM bank, and wave s only needs dm-e1
                # tiles 0..s — so neither the CAST cadence nor the dm-e1
                # arrival tail stalls the PE (measured 0.8us residual at
                # the A0->A1 boundary, vs 2.4us for kp-major order).
                xdt[ei] = inter.tile([P, nKD2, 2, C], F8, tag="xdt",
                                     name=f"xdt{ei}")
                pss = [psp.tile([P, NF], F32, tag="ps", name=f"psa{ei}_{i}")
                       for i in range(2 * nMD)]

                def mm(kp, b):
                    mc, ncc = b // 2, b % 2
                    dmt = dm_t[(ei, kp)]
                    rhs = (dmt[:, ncc] if ei == 0 and kp < 3
                           else dmt[:, :, ncc * NF:(ncc + 1) * NF])
                    nc.tensor.matmul(
                        pss[b][:], x_sb[:, kp, :, mc * P:(mc + 1) * P],
                        rhs,
                        start=(kp == 0), stop=(kp == nKP - 1),
                        perf_mode=DR)

                if not diagonal:
                    for kp in range(nKP):
                        for b in BSEQ:
                            mm(kp, b)
                else:
                    for s in range(nKP + 2 * nMD - 1):
                        for j in range(2 * nMD):
                            kp = s - j
                            if 0 <= kp < nKP:
                                mm(kp, BSEQ[j])
                # PSUM->SBUF casts in BSEQ (= stop) order.  A0's casts
                # alternate vector/scalar — the serial 0.68us/cast DVE
                # chain otherwise trails A1's diagonal ramp by 0.7-2.4us
                # every run (waves 3-6 stall on bank frees).  This only
                # works because the cmt doorbells are emitted AFTER
                # stage_A(0), leaving the scalar engine free at 24-27us.
                # A1's casts stay on vector (B1's deadline is loose and
                # scalar is running B0's Gelu activations by then).
                for idx, b in enumerate(BSEQ):
                    mc, ncc = b // 2, b % 2
                    dst = xdt[ei][:, mc // 2, mc % 2,
                                  ncc * NF:(ncc + 1) * NF]
                    if ei == 0 and idx % 2 == 1:
                        nc.scalar.activation(
                            dst, pss[b][:],
                            mybir.ActivationFunctionType.Copy)
                    else:
                        nc.vector.tensor_copy(dst, pss[b][:])

            ht = {}

            def stage_B(ei):
                # hT[HE, C] = gelu(w18^T xdT + beta), fp8 DR.
                ht[ei] = inter.tile([P, nKH2, 2, C], F8, tag="ht",
                                    name=f"ht{ei}")
                for ncc in range(2):
                    sl = slice(ncc * NF, (ncc + 1) * NF)
                    for mh in range(nMH):
                        ps0 = psp.tile([P, NF], F32, tag="ps")
                        for kd2 in range(nKD2):
                            nc.tensor.matmul(
                                ps0[:],
                                w1_sb[:, ei, kd2, :, mh * P:(mh + 1) * P],
                                xdt[ei][:, kd2, :, sl],
                                start=(kd2 == 0), stop=(kd2 == nKD2 - 1),
                                perf_mode=DR)
                        bia = beta_sb[:, ei * nMH + mh:ei * nMH + mh + 1]
                        nc.scalar.activation(ht[ei][:, mh // 2, mh % 2, sl],
                                             ps0[:], GELU, bias=bia)

            y_tiles = {}

            def stage_C(ei):
                # y[C, O] = hT^T w28, fp8 DR (DoubleRow plane layout for
                # phase D: row c = cp*256 + i*128 + p).
                y_sb = yp.tile([P, nCP, 2, O], F8, tag="y")
                for cc in range(nCC):
                    ps = psp.tile([P, NF], F32, tag="ps")
                    for kh2 in range(nKH2):
                        nc.tensor.matmul(
                            ps[:],
                            ht[ei][:, kh2, :, cc * P:(cc + 1) * P],
                            w2_sb[:, ei, kh2, :, :],
                            start=(kh2 == 0), stop=(kh2 == nKH2 - 1),
                            perf_mode=DR)
                    # split copies across vector/scalar: the serial 8-copy
                    # DVE chain otherwise extends past phase D's start in
                    # the scheduler's timeline and inflates the drain's
                    # semaphore wait targets
                    if cc % 2 == 0:
                        nc.vector.tensor_copy(y_sb[:, cc // 2, cc % 2, :],
                                              ps[:])
                    else:
                        nc.scalar.activation(
                            y_sb[:, cc // 2, cc % 2, :], ps[:],
                            mybir.ActivationFunctionType.Copy)
                y_tiles[ei] = y_sb

            # Stage order: every PSUM-copy / activation dependency gets a
            # full matmul stage of slack before its consumer (A0's copies
            # hide under A1, B0's activations under C0, etc).
            stage_A(0, diagonal=False)
            for kp in range(nCP):
                nc.scalar.dma_start(cmt_t[(0, kp)][:], cmt[0, kp])
            for kp in range(nCP):
                nc.sync.dma_start(cmt_t[(1, kp)][:], cmt[1, kp])
            stage_A(1, diagonal=True)
            stage_B(0)
            stage_C(0)
            stage_B(1)
            stage_C(1)

            # ---- phase D: pout[T, O] = sum_e cmT_e^T y_e (fp8 DR) ----
            for mt in range(nMT):
                ps = psp.tile([P, NF], F32, tag="ps")
                idx = 0
                for ei in range(2):
                    for kp in range(nCP):
                        nc.tensor.matmul(ps[:],
                                         cmt_t[(ei, kp)][:, :, mt * P:(mt + 1) * P],
                                         y_tiles[ei][:, kp, :, :],
                                         start=(idx == 0), stop=(idx == 7),
                                         perf_mode=DR)
                        idx += 1
                ot = outp.tile([P, O], BF16, tag="out")
                # alternate copy engines: vector is busy with C1's y-copies
                # when the drain starts, which otherwise delays it ~5us.
                # Queue routing keeps both HWDGE queues EMPTY when the last
                # chunk's DMAs ring: mt<=12 drains on sync only (74 GB/s
                # demand fits one queue), mt=13/14 go to scalar/sync, and
                # mt=15 is copied in one vector CAST then split across both
                # queues — each 64KB half hits an idle queue.
                if mt == nMT - 1:
                    # final chunk is the end-of-kernel critical chain:
                    # column-split the copy across both PSUM-capable
                    # engines and ship each half from its own (idle) queue
                    # as soon as its copy lands.
                    # asymmetric 320/192 column split: the scalar path
                    # starts its cast ~0.5us after vector (engine
                    # busy-ness) and rings its own doorbell, so balancing
                    # completion times puts more columns on vector/sync.
                    spl = 320
                    nc.vector.tensor_copy(ot[:, 0:spl], ps[:, 0:spl])
                    nc.sync.dma_start(pout[mt * P:(mt + 1) * P, 0:spl],
                                      ot[:, 0:spl])
                    nc.scalar.activation(ot[:, spl:O], ps[:, spl:O],
                                         mybir.ActivationFunctionType.Copy)
                    nc.scalar.dma_start(pout[mt * P:(mt + 1) * P, spl:O],
                                        ot[:, spl:O])
                    continue
                if mt % 2 == 0:
                    nc.scalar.activation(ot[:], ps[:],
                                         mybir.ActivationFunctionType.Copy)
                else:
                    nc.vector.tensor_copy(ot[:], ps[:])
                if mt == nMT - 3:
                    nc.scalar.dma_start(pout[mt * P:(mt + 1) * P, :], ot[:])
                else:
                    nc.sync.dma_start(pout[mt * P:(mt + 1) * P, :], ot[:])

    nc.compile()
    return nc


def get_nc():
    global _NC
    if _NC is None:
        _NC = _build()
    return _NC


def make_in_maps(x, dispatch_mask, combine_array, w1, b1, w2):
    f8 = ml_dtypes.float8_e4m3
    in_maps = []
    # x in partition-major DR plane layout [P, nKP, 2, D] (= x_sb's exact
    # SBUF layout, so ranged pair-loads are clean fat-line DMAs),
    # t = kp*256 + i*128 + p
    x8_by_b = [
        np.ascontiguousarray(
            x[b].reshape(nKP, 2, P, D).transpose(2, 0, 1, 3)).astype(f8)
        for b in range(B)]
    w18 = w1.astype(f8)
    w28 = w2.astype(f8)
    for m in range(8):
        b, g = m // 2, m % 2
        es = slice(2 * g, 2 * g + 2)
        # dm (shifted), t = kp*256 + i*128 + p.  Tile (e0,kp0) ships
        # h-split as dm0 [2(ncc), P, 2(i), 512] (each half one contiguous
        # 128KB DMA — its h0 start-gates the matmul stream); the other 15
        # tiles ship whole as dmr [15, P, 2, C] (256KB entries have ~15%
        # better effective queue throughput than halves).
        dm_f = (np.transpose(dispatch_mask[b, :, es, :], (1, 0, 2))
                - DM_SHIFT).reshape(2, nKP, 2, P, C)
        dm_tile = dm_f.transpose(0, 1, 3, 2, 4)         # [2, nKP, P, 2, C]
        dm0_s = np.ascontiguousarray(
            dm_tile[0, 0:3].reshape(3, P, 2, 2, NF).transpose(0, 3, 1, 2, 4)
        ).astype(f8)                                    # [3, 2, P, 2, NF]
        dmr_s = np.ascontiguousarray(
            np.concatenate([dm_tile[0, 3:], dm_tile[1]], axis=0)
        ).astype(f8)                                    # [13, P, 2, C]
        # cmT -> [2, nCP, P, 2, T], c = cp*256 + i*128 + p
        cmt_s = np.ascontiguousarray(
            np.transpose(combine_array[b, :, es, :], (1, 2, 0))
            .reshape(2, nCP, 2, P, T).transpose(0, 1, 3, 2, 4)).astype(f8)
        # w1 -> [P, 2, nKD2, 2, HE], d = kd2*256 + i*128 + p
        w1_s = np.ascontiguousarray(
            w18[es].reshape(2, nKD2, 2, P, HE).transpose(3, 0, 1, 2, 4))
        # w2 -> [P, 2, nKH2, 2, O], h' = kh2*256 + i*128 + p
        w2_s = np.ascontiguousarray(
            w28[es].reshape(2, nKH2, 2, P, O).transpose(3, 0, 1, 2, 4))
        # beta = w1^T v + b1 in fp32 with EXACT x and w1 (kills the rank-1
        # component of the x / w1 fp8 quantization errors)
        v = DM_SHIFT * x[b].sum(axis=0)                      # [D]
        beta = np.einsum("edh,d->eh", w1[es], v) + b1[es]    # [2, HE]
        beta_s = np.ascontiguousarray(
            beta.reshape(2, nMH, P).transpose(2, 0, 1).reshape(P, 2 * nMH))
        in_maps.append({
            "xb": x8_by_b[b],
            "dm0": dm0_s,
            "dmr": dmr_s,
            "cmt": cmt_s,
            "w1": w1_s,
            "w2": w2_s,
            "beta": beta_s.astype(np.float32),
        })
    return in_maps


def _norm_cdf(z):
    from math import erf
    return 0.5 * (1.0 + np.array([erf(v / np.sqrt(2.0)) for v in z],
                                 dtype=np.float64))


def kernel(x, dispatch_mask, combine_array, w1, b1, w2, b2):
    nc = get_nc()
    x, dispatch_mask, combine_array, w1, b1, w2 = (
        np.asarray(a, dtype=np.float32)
        for a in (x, dispatch_mask, combine_array, w1, b1, w2))
    in_maps = make_in_maps(x, dispatch_mask, combine_array, w1, b1, w2)
    res = bass_utils.run_bass_kernel_spmd(nc, in_maps, core_ids=list(range(8)))
    b2f = np.asarray(b2, dtype=np.float32)
    f8 = ml_dtypes.float8_e4m3
    w1q = w1.astype(f8).astype(np.float32)
    w2q = w2.astype(f8).astype(np.float32)
    ew2 = w2 - w2q                                           # [E, HE, O]
    xq = x.astype(f8).astype(np.float32)
    uq = (dispatch_mask - DM_SHIFT).astype(f8).astype(np.float32)
    out = np.empty((B, T, O), dtype=np.float32)
    for b in range(B):
        p0 = np.asarray(res.results[2 * b]["pout"], dtype=np.float32)
        p1 = np.asarray(res.results[2 * b + 1]["pout"], dtype=np.float32)
        acc = p0 + p1 + b2f
        # w2-quantization rank-1 correction per expert:
        #   out += outer(rowsum_cm, mu_h @ ew2)  with mu_h = E_c[h] estimated
        # analytically: pre-GELU activations are ~N(beta, s2) across c, so
        # mu_h = E[gelu(N(beta, s2))] in closed form (Gaussian integral).
        rs_cm = combine_array[b].sum(axis=2)                 # [T, E]
        v = DM_SHIFT * x[b].sum(axis=0)
        vu = uq[b].var(axis=2)                               # [T, E]
        for e in range(E):
            beta = (w1[e].T @ v + b1[e]).astype(np.float64)  # [HE]
            s2 = (w1q[e] ** 2).T @ ((xq[b] ** 2).T @ vu[:, e])
            s2 = s2.astype(np.float64)
            zr = beta / np.sqrt(1.0 + s2)
            phi = np.exp(-0.5 * zr * zr) / np.sqrt(2.0 * np.pi)
            mu = beta * _norm_cdf(zr) + s2 / np.sqrt(1.0 + s2) * phi
            acc += np.outer(rs_cm[:, e],
                            mu.astype(np.float32) @ ew2[e])
        out[b] = acc
    return out

